# revision 42
# baseline (speedup 1.0000x reference)
"""Decoder self-attention on 8 TRN2 NeuronCores.

Sharding: data-parallel over batch (2) x tensor-parallel over heads (4 groups
of 4 heads).  Core c handles batch c//4, heads 4*(c%4) .. 4*(c%4)+3.
Each core computes q/k/v projections for its head group, causal-masked
softmax attention, and its partial contribution ctx_g @ wo_g.  The host sums
the 4 partials per batch (row-parallel wo reduction done host-side).

Dtype strategy (rel-err budget 2e-2; measured ~1.2e-2):
  q/k/v/wo matmuls: bf16 (x and weights cast host-side)
  qT/kT storage   : fp8e4m3 (single quantization of the f32 PSUM result)
  S = K^T Q       : fp8 DoubleRow with a zeroed second subtile (head dim is
                    only 64), 2x rate over bf16
  PV              : bf16 (accuracy-critical path)
  softmax         : exp on ACT (f32 PSUM in, bf16 out), rowsum via an
                    appended ones-column in v_aug, normalize on DVE/Pool

Device data layout:
  xTb     [H=1024, L]  input.T bf16, DMA'd in key-blocks of 256 so v/q/k
                       projection units unblock progressively
  qT8,kT8 [128, c2, sub, L] fp8; sub1 = zeros (DoubleRow padding)
  S       [lk 128, lq] logit strips in PSUM
  expS    [lk 128, lq] exp'd strips in SBUF (bf16)
  v_aug   [128, lt, 4*65] bf16 v rows + ones column per head
  ctxT    [128, 2, L] bf16 normalized context, partitions = ctx dim
  out     [L, 1024] bf16 partial output (host upcasts + sums)
"""

import os
from contextlib import ExitStack

import ml_dtypes
import numpy as np

import concourse.tile as tile
from concourse import bacc, mybir
from concourse.bass_utils import run_bass_kernel_spmd

f32 = mybir.dt.float32
bf16 = mybir.dt.bfloat16
fp8 = mybir.dt.float8e4
DR = mybir.MatmulPerfMode.DoubleRow

H = 1024          # hidden dim
WG = 256          # weight-column group per core (4 heads x 64)
NH = 4            # heads per core
HD = 64           # head dim
INV_SQRT_D = 1.0 / 32.0  # 1/sqrt(1024)

_PROGRAM_CACHE = {}
LAST_RESULT = None
MM_LABELS = []  # emission-order matmul labels (analysis aid)


def build_program(L=2048, QCH=1024, repeat=1, opts=None):
    assert L % QCH == 0 and QCH % 512 == 0 and QCH <= 1024
    opts = dict(opts or {})
    NLT = L // 128
    NQH = L // QCH

    nc = bacc.Bacc("TRN2", target_bir_lowering=False, debug=False)
    xTb_d = nc.dram_tensor("xTb", [H, L], bf16, kind="ExternalInput").ap()
    wqb_d = nc.dram_tensor("wqb", [H, WG], bf16, kind="ExternalInput").ap()
    wkb_d = nc.dram_tensor("wkb", [H, WG], bf16, kind="ExternalInput").ap()
    wvb_d = nc.dram_tensor("wvb", [H, WG], bf16, kind="ExternalInput").ap()
    wob_d = nc.dram_tensor("wob", [WG, H], bf16, kind="ExternalInput").ap()
    lnm_d = nc.dram_tensor("lnm", [L], f32, kind="ExternalInput").ap()
    tri_d = nc.dram_tensor("tri", [128, 128], bf16, kind="ExternalInput").ap()
    onescol_d = nc.dram_tensor(
        "onescol", [128, NLT * NH], bf16, kind="ExternalInput"
    ).ap()
    out_d = nc.dram_tensor("out", [L, H], bf16, kind="ExternalOutput").ap()

    with ExitStack() as ctx:
        tc = ctx.enter_context(tile.TileContext(nc))

        persist = ctx.enter_context(tc.tile_pool(name="persist", bufs=1))
        qT8 = persist.tile([128, 2, 2, L], fp8, tag="qT8")
        kT8 = persist.tile([128, 2, 2, L], fp8, tag="kT8")
        v_aug = persist.tile([128, NLT, NH * (HD + 1)], bf16, tag="vaug")
        ctxT0 = persist.tile([128, L], bf16, tag="ctxT0")
        ctxT1 = persist.tile([128, L], bf16, tag="ctxT1")
        ctxT = [ctxT0, ctxT1]
        wo_sb = persist.tile([128, 2, H], bf16, tag="wo")
        lnm_sb = persist.tile([128, NLT], f32, tag="lnm")
        tri_sb = persist.tile([128, 128], bf16, tag="tri")

        env = locals()
        for _rep in range(repeat):
            _build_body(nc, tc, ctx, env)

    nc.compile()
    return nc


def _build_body(nc, tc, ctx, env):
    L = env["L"]; QCH = env["QCH"]; NLT = env["NLT"]; NQH = env["NQH"]
    xTb_d = env["xTb_d"]
    wqb_d = env["wqb_d"]; wkb_d = env["wkb_d"]; wvb_d = env["wvb_d"]
    wob_d = env["wob_d"]; lnm_d = env["lnm_d"]
    tri_d = env["tri_d"]; onescol_d = env["onescol_d"]; out_d = env["out_d"]
    qT8 = env["qT8"]; kT8 = env["kT8"]; v_aug = env["v_aug"]
    ctxT = env["ctxT"]
    wo_sb = env["wo_sb"]; lnm_sb = env["lnm_sb"]; tri_sb = env["tri_sb"]
    opts = env["opts"]

    LA = opts.get("lookahead", 20)    # S/exp units emitted ahead of PV
    EXPP_BUFS = opts.get("expp_bufs", LA + 2)
    # v_aug[kt] must be emitted no later than attention unit kt (PV of
    # (qh0,h0,kt) is unit kt), so v_step=1 with v_start=0 is the latest safe
    V_START = opts.get("v_start", 0)  # att-unit index where v injection begins
    V_STEP = opts.get("v_step", 1)    # att units between v injections
    QK1_START = opts.get("qk1_start", 16)
    QK1_STEP = opts.get("qk1_step", 2)
    WO0_STEP = opts.get("wo0_step", 1)
    TAIL_SPLIT = opts.get("tail_split", True)

    # ---- transient input pools ----
    trans = ctx.enter_context(tc.tile_pool(name="transient", bufs=1))
    xTb_sb = trans.tile([128, 8, L], bf16, tag="xTb")
    wqb_sb = trans.tile([128, 8, WG], bf16, tag="wqb")
    wkb_sb = trans.tile([128, 8, WG], bf16, tag="wkb")
    wvb_sb = trans.tile([128, 8, WG], bf16, tag="wvb")

    # DMAs in consumption order: q(qc0) needs wq + xTb blocks 0-1, then
    # k(qc0) needs wk, v0 needs wv -- interleave weights into the block
    # stream so the first projection unblocks at ~4.7us instead of ~10us.
    # xTb in key-blocks of 256: q/k unit qc needs blocks 2qc,2qc+1 and
    # v-proj l-tile lt needs block lt//2, so PE streams behind the DMA.
    xTb_r = xTb_d.rearrange("(c p) l -> p c l", p=128)

    def xtb_block(kb):
        s = slice(256 * kb, 256 * kb + 256)
        nc.sync.dma_start(out=xTb_sb[:, :, s], in_=xTb_r[:, :, s])

    nc.sync.dma_start(out=wqb_sb, in_=wqb_d.rearrange("(c p) d -> p c d", p=128))
    xtb_block(0)
    xtb_block(1)
    nc.sync.dma_start(out=wkb_sb, in_=wkb_d.rearrange("(c p) d -> p c d", p=128))
    nc.sync.dma_start(out=wvb_sb, in_=wvb_d.rearrange("(c p) d -> p c d", p=128))
    for kb in range(2, L // 256):
        xtb_block(kb)
    nc.sync.dma_start(out=wo_sb, in_=wob_d.rearrange("(c p) d -> p c d", p=128))
    nc.sync.dma_start(out=lnm_sb, in_=lnm_d.rearrange("(t p) -> p t", p=128))
    nc.sync.dma_start(out=tri_sb, in_=tri_d)
    ones_cols = v_aug.rearrange("p t (h j) -> p t h j", j=HD + 1)[:, :, :, HD : HD + 1]
    nc.sync.dma_start(
        out=ones_cols,
        in_=onescol_d.rearrange("p (t h) -> p t h", h=NH)[:, :, :, None],
    )
    # DoubleRow zero subtiles (Pool engine; SBUF-only)
    for t8 in (qT8, kT8):
        for c2 in range(2):
            nc.gpsimd.memset(t8[:, c2, 1, :], 0.0)

    # ---- pools ----
    # PSUM budget (8 banks): s_ps 2x[128,1024]=4, ctx_ps 1x[128,1024]=2,
    # small_ps 2x[128,512]=2 (shared by qk-proj, v-proj, wo)
    s_ps = ctx.enter_context(
        tc.tile_pool(name="s_ps", bufs=opts.get("s_bufs", 2), space="PSUM"))
    ctx_ps = ctx.enter_context(
        tc.tile_pool(name="ctx_ps", bufs=opts.get("ctx_bufs", 1), space="PSUM"))
    small_ps = ctx.enter_context(
        tc.tile_pool(name="small_ps", bufs=opts.get("small_bufs", 2), space="PSUM"))
    expp = ctx.enter_context(tc.tile_pool(name="expp", bufs=EXPP_BUFS))
    rp = ctx.enter_context(tc.tile_pool(name="rp", bufs=opts.get("rp_bufs", 3)))
    ctxsbp = ctx.enter_context(tc.tile_pool(name="ctxsbp", bufs=opts.get("ctxsb_bufs", 2)))
    outp = ctx.enter_context(tc.tile_pool(name="outp", bufs=opts.get("outp_bufs", 8)))

    # PE warmup: junk matmuls on the first-arrived weight tile keep the PE
    # pstate ramped while xTb streams in (results overwritten by real work)
    for _w in range(opts.get("warmup", 0)):
        MM_LABELS.append(f"warm{_w}")
        nc.tensor.matmul(
            s_ps.tile([128, 128], f32, tag="S", name="warm"),
            lhsT=wqb_sb[:, 0, 0:128],
            rhs=wqb_sb[:, 1, 0:128],
            start=True,
            stop=True,
        )

    emitted_proj = set()   # ("q"|"k", c2, qc) and ("v", lt)

    # ---- projection emitters ----
    def emit_qk(which, c2, qc):
        emitted_proj.add((which, c2, qc))
        # bf16 matmul, fp8 storage of the result (single quantization)
        wt = wqb_sb if which == "q" else wkb_sb
        dst = qT8 if which == "q" else kT8
        ps = small_ps.tile([128, 512], f32, tag="sp", name="qkps")
        for hc in range(8):
            MM_LABELS.append(f"{which}{c2}{qc}.{hc}")
            nc.tensor.matmul(
                ps,
                lhsT=wt[:, hc, 128 * c2 : 128 * c2 + 128],
                rhs=xTb_sb[:, hc, 512 * qc : 512 * qc + 512],
                start=(hc == 0),
                stop=(hc == 7),
            )
        nc.vector.tensor_copy(
            out=dst[:, c2, 0, 512 * qc : 512 * qc + 512], in_=ps)

    def emit_v(lt):
        emitted_proj.add(("v", lt))
        vtile = small_ps.tile([128, 512], f32, tag="sp", name="vps")
        vps = vtile[:, 0:256]
        for hc in range(8):
            MM_LABELS.append(f"v{lt}.{hc}")
            nc.tensor.matmul(
                vps,
                lhsT=xTb_sb[:, hc, 128 * lt : 128 * lt + 128],
                rhs=wvb_sb[:, hc, :],
                start=(hc == 0),
                stop=(hc == 7),
            )
        dest = v_aug[:, lt, :].rearrange("p (h j) -> p h j", j=HD + 1)[:, :, 0:HD]
        nc.vector.tensor_copy(out=dest, in_=vps.rearrange("p (h j) -> p h j", j=HD))

    # ---- attention units: rotated head order so c2=1 q/k can be late ----
    units = []
    for hb in range(2):              # c2 half
        for qh in range(NQH):
            for h in (2 * hb, 2 * hb + 1):
                q0 = qh * QCH
                ktmax = (q0 + QCH - 1) // 128
                for kt in range(ktmax + 1):
                    o = max(0, 128 * kt - q0)
                    chunks = []
                    n0 = o
                    while n0 < QCH:
                        n1 = min(QCH, (n0 // 512 + 1) * 512)
                        chunks.append((n0, n1, (q0 + n1 - 1) // 128))
                        n0 = n1
                    units.append((qh, h, kt, o, chunks))

    exp_tiles = {}

    SHORT_SMALL = opts.get("short_small", False)

    def emit_se(i):
        qh, h, kt, o, chunks = units[i]
        q0 = qh * QCH
        p0 = HD * (h % 2)
        c2 = h // 2
        # short diagonal units (S region within one 512-col bank) borrow the
        # small pool so long units get the full s_ps ring to themselves
        short = SHORT_SMALL and o >= QCH - 512
        if short:
            S = small_ps.tile([128, 512], f32, tag="sp", name="Ss")
            base = QCH - 512
        else:
            S = s_ps.tile([128, QCH], f32, tag="S", name="S")
            base = 0
        for (n0, n1, _) in chunks:
            MM_LABELS.append(f"S.q{qh}h{h}k{kt}.{n0}")
            nc.tensor.matmul(
                S[:, n0 - base : n1 - base],
                lhsT=kT8[p0 : p0 + HD, c2, :, 128 * kt : 128 * kt + 128],
                rhs=qT8[p0 : p0 + HD, c2, :, q0 + n0 : q0 + n1],
                start=True,
                stop=True,
                perf_mode=DR,
            )
        expS = expp.tile([128, QCH], bf16, tag="expS", name="expS")
        nc.scalar.activation(
            out=expS[:, o:QCH],
            in_=S[:, o - base : QCH - base],
            func=mybir.ActivationFunctionType.Exp,
            scale=INV_SQRT_D,
            bias=lnm_sb[:, kt : kt + 1],
        )
        if 128 * kt >= q0:
            if opts.get("tri_engine", "vector") == "pool":
                nc.gpsimd.tensor_mul(
                    out=expS[:, o : o + 128], in0=expS[:, o : o + 128],
                    in1=tri_sb)
            else:
                nc.vector.tensor_mul(
                    out=expS[:, o : o + 128], in0=expS[:, o : o + 128],
                    in1=tri_sb)
        exp_tiles[i] = expS

    def emit_finalize(qh, h, j0, j1, last=False):
        # normalize ctx columns [128*j0, 128*j1) of chunk qh for head h
        q0 = qh * QCH
        p0 = HD * (h % 2)
        c2 = h // 2
        n0, n1 = 128 * j0, 128 * j1
        w = n1 - n0
        ctx_t = cur_ctx[0]
        if last:
            # final bank: no next head waits on this PSUM tile -- normalize
            # straight out of PSUM, in 128-col pieces so the first tail wo
            # unblocks after ~0.8us instead of the full-bank chain
            for m0 in range(n0, n1, 128):
                m1 = m0 + 128
                mw = 128
                r = rp.tile([1, mw], bf16, tag="r", name="r")
                with nc.allow_low_precision(reason="softmax denom recip bf16"):
                    nc.vector.reciprocal(r, ctx_t[HD : HD + 1, m0:m1])
                r64 = rp.tile([HD, mw], bf16, tag="r64", name="r64")
                nc.gpsimd.partition_broadcast(r64, r)
                nc.vector.tensor_mul(
                    out=ctxT[c2][p0 : p0 + HD, q0 + m0 : q0 + m1],
                    in0=ctx_t[0:HD, m0:m1],
                    in1=r64,
                )
            return
        # copy PSUM->SBUF first (releases the ctx accumulator for the next
        # head), then normalize the SBUF copy; bf16 end-to-end so the DVE
        # 2x two-byte mode applies to recip and mul
        ctx_sb = ctxsbp.tile([HD + 1, w], bf16, tag="ctxsb", name="ctxsb")
        nc.vector.tensor_copy(out=ctx_sb, in_=ctx_t[0 : HD + 1, n0:n1])
        r = rp.tile([1, w], bf16, tag="r", name="r")
        with nc.allow_low_precision(reason="softmax denom recip in bf16"):
            nc.vector.reciprocal(r, ctx_sb[HD : HD + 1, :])
        r64 = rp.tile([HD, w], bf16, tag="r64", name="r64")
        nc.gpsimd.partition_broadcast(r64, r)
        nc.vector.tensor_mul(
            out=ctxT[c2][p0 : p0 + HD, q0 + n0 : q0 + n1],
            in0=ctx_sb[0:HD, :],
            in1=r64,
        )

    osb_tiles = {}

    def emit_wo(qh, lt, n2, evac="vector"):
        # both 512-col halves of a row-tile stage into one [128,1024] SBUF
        # tile and go out in a single 2KB-per-partition DMA (halves HWDGE
        # dispatch cost at the tail)
        q0 = qh * QCH
        l0 = q0 + 128 * lt
        wps = small_ps.tile([128, 512], f32, tag="sp", name="wops")
        for cc in range(2):
            MM_LABELS.append(f"wo.q{qh}l{lt}n{n2}.{cc}")
            nc.tensor.matmul(
                wps,
                lhsT=ctxT[cc][:, l0 : l0 + 128],
                rhs=wo_sb[:, cc, 512 * n2 : 512 * n2 + 512],
                start=(cc == 0),
                stop=(cc == 1),
            )
        if (qh, lt) not in osb_tiles:
            osb_tiles[(qh, lt)] = outp.tile([128, H], bf16, tag="osb", name="osb")
        osb = osb_tiles[(qh, lt)]
        if evac == "scalar":
            nc.scalar.copy(out=osb[:, 512 * n2 : 512 * n2 + 512], in_=wps)
        else:
            nc.vector.tensor_copy(out=osb[:, 512 * n2 : 512 * n2 + 512], in_=wps)
        if n2 == 1:
            del osb_tiles[(qh, lt)]
            nc.sync.dma_start(out=out_d[l0 : l0 + 128, :], in_=osb)

    # ---- build interleaved schedule ----
    # background injections keyed by attention-unit index
    inject = {i: [] for i in range(len(units) + 1)}
    for j in range(NLT):
        inject[min(V_START + j * V_STEP, len(units))].append(("v", j))
    k = 0
    for c2 in range(2):
        for qc in range(4):
            for which in ("q", "k"):
                if c2 == 0:
                    continue
                inject[min(QK1_START + k * QK1_STEP, len(units))].append(
                    ("qk", which, 1, qc))
                k += 1
    # wo(qh) becomes ready when the (qh, h3) group ends; spread its units
    # into the stream after that point (the last qh is handled by the tail)
    last_qh = NQH - 1
    h3_end = {}
    for i, (qh, h, kt, o, chunks) in enumerate(units):
        if h == 3:
            h3_end[qh] = i + 1
    for qh in range(NQH - 1):
        wo_units = [(qh, lt, n2) for lt in range(QCH // 128) for n2 in range(2)]
        for j, wu in enumerate(wo_units):
            ev = "scalar" if (opts.get("wo0_alt", False) and j % 2) else "vector"
            inject[min(h3_end[qh] + j * WO0_STEP, len(units))].append(
                ("wo", wu[0], wu[1], wu[2], ev))

    cur_ctx = [None]
    se_cursor = [0]
    tail_wo = []


    def se_ready(i):
        # S(i) may only be emitted after the projections it reads are emitted
        qh, h, kt, o, chunks = units[i]
        c2 = h // 2
        for qc in range(qh * (QCH // 512), (qh + 1) * (QCH // 512)):
            if ("q", c2, qc) not in emitted_proj:
                return False
        return ("k", c2, kt // 4) in emitted_proj

    def pump_se(limit):
        while se_cursor[0] <= min(limit, len(units) - 1) and se_ready(se_cursor[0]):
            emit_se(se_cursor[0])
            se_cursor[0] += 1

    # minimal projection prefix: S(qh0,h0,kt0..3) needs q qc0-1 + k qc0;
    # everything else streams into the attention loop below
    if opts.get("full_prefix", True):
        PRE = [("q", 0, qc) for qc in range(4)] + [("k", 0, qc) for qc in range(4)]
    else:
        PRE = [("q", 0, 0), ("k", 0, 0), ("q", 0, 1)]
    for which, c2, qc in PRE:
        emit_qk(which, c2, qc)
    pre_set = set(PRE)
    # remaining c2=0 projections injected early (kt order needs k qc asc)
    rest = [it for it in
            [("k", 0, 1), ("q", 0, 2), ("k", 0, 2), ("q", 0, 3), ("k", 0, 3)]
            if it not in pre_set]
    QK0_START = opts.get("qk0_start", 1)
    QK0_STEP = opts.get("qk0_step", 2)
    for j, it in enumerate(rest):
        inject[min(QK0_START + j * QK0_STEP, len(units))].append(("qk",) + it)

    PUMP_FIRST = opts.get("pump_first", True)
    for i, (qh, h, kt, o, chunks) in enumerate(units):
        if PUMP_FIRST:
            pump_se(i + LA)
        for item in inject.get(i, []):
            if item[0] == "v":
                emit_v(item[1])
            elif item[0] == "qk":
                emit_qk(item[1], item[2], item[3])
            elif item[0] == "wo":
                emit_wo(item[1], item[2], item[3], evac=item[4])
        pump_se(i + LA)
        q0 = qh * QCH
        c2 = h // 2
        if kt == 0:
            cur_ctx[0] = ctx_ps.tile([128, QCH], f32, tag="ctx", name="ctx")
        expS = exp_tiles.pop(i)
        for (n0, n1, kstop) in chunks:
            MM_LABELS.append(f"PV.q{qh}h{h}k{kt}.{n0}")
            nc.tensor.matmul(
                cur_ctx[0][0 : HD + 1, n0:n1],
                lhsT=v_aug[:, kt, (HD + 1) * h : (HD + 1) * (h + 1)],
                rhs=expS[:, n0:n1],
                start=(kt == 0),
                stop=(kt == kstop),
            )
        ktmax = (q0 + QCH - 1) // 128
        tail_head = TAIL_SPLIT and qh == last_qh and h == 3
        if tail_head:
            # drain queued wo BEFORE this unit's finalize: drains then lag
            # the finalize chain by one unit so its DVE mul has finished
            for _ in range(opts.get("tail_wo_rate", 2)):
                if tail_wo:
                    tqh, tlt, tn2 = tail_wo.pop(0)
                    emit_wo(tqh, tlt, tn2,
                            evac=("scalar" if tn2 else "vector"))
            # PSUM accumulation groups close per 2KB bank (512 f32 cols):
            # finalize bank b as soon as its last PV lands, then trickle its
            # wo units between the remaining PV units so the PSUM ring and
            # evac engines pipeline instead of stalling PE back-to-back
            for b in range(QCH // 512):
                if kt == (q0 + 512 * (b + 1) - 1) // 128:
                    emit_finalize(qh, h, 4 * b, 4 * b + 4,
                                  last=(b == QCH // 512 - 1))
                    for lt in range(4 * b, 4 * b + 4):
                        for n2 in range(2):
                            tail_wo.append((qh, lt, n2))
        elif kt == ktmax:
            emit_finalize(qh, h, 0, QCH // 128)
            if qh == last_qh and h == 3:
                for lt in range(QCH // 128):
                    for n2 in range(2):
                        emit_wo(qh, lt, n2,
                                evac=("scalar" if n2 else "vector"))
    while tail_wo:
        tqh, tlt, tn2 = tail_wo.pop(0)
        emit_wo(tqh, tlt, tn2, evac=("scalar" if tn2 else "vector"))
    for item in inject.get(len(units), []):
        if item[0] == "v":
            emit_v(item[1])
        elif item[0] == "qk":
            emit_qk(item[1], item[2], item[3])
        elif item[0] == "wo":
            emit_wo(item[1], item[2], item[3], evac=item[4])


def _get_program(L=2048, QCH=1024):
    key = (L, QCH)
    if key not in _PROGRAM_CACHE:
        _PROGRAM_CACHE[key] = build_program(L, QCH)
    return _PROGRAM_CACHE[key]


def make_in_maps(x, am, wq, wk, wv, wo):
    B, L, _ = x.shape
    tri = np.triu(np.ones((128, 128), dtype=np.float32)).astype(ml_dtypes.bfloat16)
    in_maps = []
    for c in range(8):
        b, g = divmod(c, 4)
        cols = slice(WG * g, WG * (g + 1))
        m = am[b].astype(np.float32)
        lnm = np.where(m > 0, np.log(np.maximum(m, 1e-38)), -1e38).astype(np.float32)
        onescol = np.ones((128, (L // 128) * 4), dtype=ml_dtypes.bfloat16)
        xT = np.ascontiguousarray(x[b].T)
        in_maps.append(
            {
                "onescol": onescol,
                "xTb": xT.astype(ml_dtypes.bfloat16),
                "wqb": np.ascontiguousarray(wq[:, cols]).astype(ml_dtypes.bfloat16),
                "wkb": np.ascontiguousarray(wk[:, cols]).astype(ml_dtypes.bfloat16),
                "wvb": np.ascontiguousarray(wv[:, cols]).astype(ml_dtypes.bfloat16),
                "wob": np.ascontiguousarray(wo[cols, :]).astype(ml_dtypes.bfloat16),
                "lnm": lnm,
                "tri": tri,
            }
        )
    return in_maps


def kernel(**inputs):
    global LAST_RESULT
    x = np.asarray(inputs["input"], dtype=np.float32)
    am = np.asarray(inputs["attention_mask"], dtype=np.float32)
    wq = np.asarray(inputs["wq"], dtype=np.float32)
    wk = np.asarray(inputs["wk"], dtype=np.float32)
    wv = np.asarray(inputs["wv"], dtype=np.float32)
    wo = np.asarray(inputs["wo"], dtype=np.float32)
    B, L, _ = x.shape

    nc = _get_program(L=L, QCH=min(1024, L))
    in_maps = make_in_maps(x, am, wq, wk, wv, wo)
    trace = os.environ.get("KERNEL_TRACE", "0") == "1"
    res = run_bass_kernel_spmd(nc, in_maps, list(range(8)), trace=trace)
    LAST_RESULT = res

    out = np.zeros((B, L, H), dtype=np.float32)
    for b in range(B):
        for g in range(4):
            out[b] += res.results[4 * b + g]["out"].astype(np.float32)
    return out


# revision 44
# speedup vs baseline: 1.0015x; 1.0015x over previous
"""Decoder self-attention on 8 TRN2 NeuronCores.

Sharding: data-parallel over batch (2) x tensor-parallel over heads (4 groups
of 4 heads).  Core c handles batch c//4, heads 4*(c%4) .. 4*(c%4)+3.
Each core computes q/k/v projections for its head group, causal-masked
softmax attention, and its partial contribution ctx_g @ wo_g.  The host sums
the 4 partials per batch (row-parallel wo reduction done host-side).

Dtype strategy (rel-err budget 2e-2; measured ~1.2e-2):
  q/k/v/wo matmuls: bf16 (x and weights cast host-side)
  qT/kT storage   : fp8e4m3 (single quantization of the f32 PSUM result)
  S = K^T Q       : fp8 DoubleRow with a zeroed second subtile (head dim is
                    only 64), 2x rate over bf16
  PV              : bf16 (accuracy-critical path)
  softmax         : exp on ACT (f32 PSUM in, bf16 out), rowsum via an
                    appended ones-column in v_aug, normalize on DVE/Pool

Device data layout:
  xTb     [H=1024, L]  input.T bf16, DMA'd in key-blocks of 256 so v/q/k
                       projection units unblock progressively
  qT8,kT8 [128, c2, sub, L] fp8; sub1 = zeros (DoubleRow padding)
  S       [lk 128, lq] logit strips in PSUM
  expS    [lk 128, lq] exp'd strips in SBUF (bf16)
  v_aug   [128, lt, 4*65] bf16 v rows + ones column per head
  ctxT    [128, 2, L] bf16 normalized context, partitions = ctx dim
  out     [L, 1024] bf16 partial output (host upcasts + sums)
"""

import os
from contextlib import ExitStack

import ml_dtypes
import numpy as np

import concourse.tile as tile
from concourse import bacc, mybir
from concourse.bass_utils import run_bass_kernel_spmd

f32 = mybir.dt.float32
bf16 = mybir.dt.bfloat16
fp8 = mybir.dt.float8e4
DR = mybir.MatmulPerfMode.DoubleRow

H = 1024          # hidden dim
WG = 256          # weight-column group per core (4 heads x 64)
NH = 4            # heads per core
HD = 64           # head dim
INV_SQRT_D = 1.0 / 32.0  # 1/sqrt(1024)

_PROGRAM_CACHE = {}
LAST_RESULT = None
MM_LABELS = []  # emission-order matmul labels (analysis aid)


def build_program(L=2048, QCH=1024, repeat=1, opts=None):
    assert L % QCH == 0 and QCH % 512 == 0 and QCH <= 1024
    opts = dict(opts or {})
    NLT = L // 128
    NQH = L // QCH

    nc = bacc.Bacc("TRN2", target_bir_lowering=False, debug=False)
    xTb_d = nc.dram_tensor("xTb", [H, L], bf16, kind="ExternalInput").ap()
    wqb_d = nc.dram_tensor("wqb", [H, WG], bf16, kind="ExternalInput").ap()
    wkb_d = nc.dram_tensor("wkb", [H, WG], bf16, kind="ExternalInput").ap()
    wvb_d = nc.dram_tensor("wvb", [H, WG], bf16, kind="ExternalInput").ap()
    wob_d = nc.dram_tensor("wob", [WG, H], bf16, kind="ExternalInput").ap()
    lnm_d = nc.dram_tensor("lnm", [L], f32, kind="ExternalInput").ap()
    tri_d = nc.dram_tensor("tri", [128, 128], bf16, kind="ExternalInput").ap()
    onescol_d = nc.dram_tensor(
        "onescol", [128, NLT * NH], bf16, kind="ExternalInput"
    ).ap()
    out_d = nc.dram_tensor("out", [L, H], bf16, kind="ExternalOutput").ap()

    with ExitStack() as ctx:
        tc = ctx.enter_context(tile.TileContext(nc))

        persist = ctx.enter_context(tc.tile_pool(name="persist", bufs=1))
        qT8 = persist.tile([128, 2, 2, L], fp8, tag="qT8")
        kT8 = persist.tile([128, 2, 2, L], fp8, tag="kT8")
        v_aug = persist.tile([128, NLT, NH * (HD + 1)], bf16, tag="vaug")
        ctxT0 = persist.tile([128, L], bf16, tag="ctxT0")
        ctxT1 = persist.tile([128, L], bf16, tag="ctxT1")
        ctxT = [ctxT0, ctxT1]
        wo_sb = persist.tile([128, 2, H], bf16, tag="wo")
        lnm_sb = persist.tile([128, NLT], f32, tag="lnm")
        tri_sb = persist.tile([128, 128], bf16, tag="tri")

        env = locals()
        for _rep in range(repeat):
            _build_body(nc, tc, ctx, env)

    nc.compile()
    return nc


def _build_body(nc, tc, ctx, env):
    L = env["L"]; QCH = env["QCH"]; NLT = env["NLT"]; NQH = env["NQH"]
    xTb_d = env["xTb_d"]
    wqb_d = env["wqb_d"]; wkb_d = env["wkb_d"]; wvb_d = env["wvb_d"]
    wob_d = env["wob_d"]; lnm_d = env["lnm_d"]
    tri_d = env["tri_d"]; onescol_d = env["onescol_d"]; out_d = env["out_d"]
    qT8 = env["qT8"]; kT8 = env["kT8"]; v_aug = env["v_aug"]
    ctxT = env["ctxT"]
    wo_sb = env["wo_sb"]; lnm_sb = env["lnm_sb"]; tri_sb = env["tri_sb"]
    opts = env["opts"]

    LA = opts.get("lookahead", 20)    # S/exp units emitted ahead of PV
    PF_MAX = opts.get("pf_max", 24)   # hb1 S/exp units prefetched into hb0
    PF_START = opts.get("pf_start", 32)
    PF_STEP = opts.get("pf_step", 1)
    EXPP_BUFS = opts.get("expp_bufs", LA + 2 + PF_MAX)
    # v_aug[kt] must be emitted no later than attention unit kt (PV of
    # (qh0,h0,kt) is unit kt), so v_step=1 with v_start=0 is the latest safe
    V_START = opts.get("v_start", 0)  # att-unit index where v injection begins
    V_STEP = opts.get("v_step", 1)    # att units between v injections
    QK1_START = opts.get("qk1_start", 16)
    QK1_STEP = opts.get("qk1_step", 2)
    WO0_STEP = opts.get("wo0_step", 1)
    TAIL_SPLIT = opts.get("tail_split", True)

    # ---- transient input pools ----
    trans = ctx.enter_context(tc.tile_pool(name="transient", bufs=1))
    xTb_sb = trans.tile([128, 8, L], bf16, tag="xTb")
    wqb_sb = trans.tile([128, 8, WG], bf16, tag="wqb")
    wkb_sb = trans.tile([128, 8, WG], bf16, tag="wkb")
    wvb_sb = trans.tile([128, 8, WG], bf16, tag="wvb")

    # DMAs in consumption order: q(qc0) needs wq + xTb blocks 0-1, then
    # k(qc0) needs wk, v0 needs wv -- interleave weights into the block
    # stream so the first projection unblocks at ~4.7us instead of ~10us.
    # xTb in key-blocks of 256: q/k unit qc needs blocks 2qc,2qc+1 and
    # v-proj l-tile lt needs block lt//2, so PE streams behind the DMA.
    xTb_r = xTb_d.rearrange("(c p) l -> p c l", p=128)

    def xtb_block(kb):
        s = slice(256 * kb, 256 * kb + 256)
        nc.sync.dma_start(out=xTb_sb[:, :, s], in_=xTb_r[:, :, s])

    nc.sync.dma_start(out=wqb_sb, in_=wqb_d.rearrange("(c p) d -> p c d", p=128))
    xtb_block(0)
    xtb_block(1)
    nc.sync.dma_start(out=wkb_sb, in_=wkb_d.rearrange("(c p) d -> p c d", p=128))
    nc.sync.dma_start(out=wvb_sb, in_=wvb_d.rearrange("(c p) d -> p c d", p=128))
    for kb in range(2, L // 256):
        xtb_block(kb)
    nc.sync.dma_start(out=wo_sb, in_=wob_d.rearrange("(c p) d -> p c d", p=128))
    nc.sync.dma_start(out=lnm_sb, in_=lnm_d.rearrange("(t p) -> p t", p=128))
    nc.sync.dma_start(out=tri_sb, in_=tri_d)
    ones_cols = v_aug.rearrange("p t (h j) -> p t h j", j=HD + 1)[:, :, :, HD : HD + 1]
    nc.sync.dma_start(
        out=ones_cols,
        in_=onescol_d.rearrange("p (t h) -> p t h", h=NH)[:, :, :, None],
    )
    # DoubleRow zero subtiles (Pool engine; SBUF-only)
    for t8 in (qT8, kT8):
        for c2 in range(2):
            nc.gpsimd.memset(t8[:, c2, 1, :], 0.0)

    # ---- pools ----
    # PSUM budget (8 banks): s_ps 2x[128,1024]=4, ctx_ps 1x[128,1024]=2,
    # small_ps 2x[128,512]=2 (shared by qk-proj, v-proj, wo)
    s_ps = ctx.enter_context(
        tc.tile_pool(name="s_ps", bufs=opts.get("s_bufs", 2), space="PSUM"))
    ctx_ps = ctx.enter_context(
        tc.tile_pool(name="ctx_ps", bufs=opts.get("ctx_bufs", 1), space="PSUM"))
    small_ps = ctx.enter_context(
        tc.tile_pool(name="small_ps", bufs=opts.get("small_bufs", 2), space="PSUM"))
    expp = ctx.enter_context(tc.tile_pool(name="expp", bufs=EXPP_BUFS))
    rp = ctx.enter_context(tc.tile_pool(name="rp", bufs=opts.get("rp_bufs", 3)))
    ctxsbp = ctx.enter_context(tc.tile_pool(name="ctxsbp", bufs=opts.get("ctxsb_bufs", 2)))
    outp = ctx.enter_context(tc.tile_pool(name="outp", bufs=opts.get("outp_bufs", 8)))

    # PE warmup: junk matmuls on the first-arrived weight tile keep the PE
    # pstate ramped while xTb streams in (results overwritten by real work)
    for _w in range(opts.get("warmup", 0)):
        MM_LABELS.append(f"warm{_w}")
        nc.tensor.matmul(
            s_ps.tile([128, 128], f32, tag="S", name="warm"),
            lhsT=wqb_sb[:, 0, 0:128],
            rhs=wqb_sb[:, 1, 0:128],
            start=True,
            stop=True,
        )

    emitted_proj = set()   # ("q"|"k", c2, qc) and ("v", lt)

    # ---- projection emitters ----
    def emit_qk(which, c2, qc):
        emitted_proj.add((which, c2, qc))
        # bf16 matmul, fp8 storage of the result (single quantization)
        wt = wqb_sb if which == "q" else wkb_sb
        dst = qT8 if which == "q" else kT8
        ps = small_ps.tile([128, 512], f32, tag="sp", name="qkps")
        for hc in range(8):
            MM_LABELS.append(f"{which}{c2}{qc}.{hc}")
            nc.tensor.matmul(
                ps,
                lhsT=wt[:, hc, 128 * c2 : 128 * c2 + 128],
                rhs=xTb_sb[:, hc, 512 * qc : 512 * qc + 512],
                start=(hc == 0),
                stop=(hc == 7),
            )
        nc.vector.tensor_copy(
            out=dst[:, c2, 0, 512 * qc : 512 * qc + 512], in_=ps)

    def emit_v(lt):
        emitted_proj.add(("v", lt))
        vtile = small_ps.tile([128, 512], f32, tag="sp", name="vps")
        vps = vtile[:, 0:256]
        for hc in range(8):
            MM_LABELS.append(f"v{lt}.{hc}")
            nc.tensor.matmul(
                vps,
                lhsT=xTb_sb[:, hc, 128 * lt : 128 * lt + 128],
                rhs=wvb_sb[:, hc, :],
                start=(hc == 0),
                stop=(hc == 7),
            )
        dest = v_aug[:, lt, :].rearrange("p (h j) -> p h j", j=HD + 1)[:, :, 0:HD]
        nc.vector.tensor_copy(out=dest, in_=vps.rearrange("p (h j) -> p h j", j=HD))

    # ---- attention units: rotated head order so c2=1 q/k can be late ----
    units = []
    for hb in range(2):              # c2 half
        for qh in range(NQH):
            for h in (2 * hb, 2 * hb + 1):
                q0 = qh * QCH
                ktmax = (q0 + QCH - 1) // 128
                for kt in range(ktmax + 1):
                    o = max(0, 128 * kt - q0)
                    chunks = []
                    n0 = o
                    while n0 < QCH:
                        n1 = min(QCH, (n0 // 512 + 1) * 512)
                        chunks.append((n0, n1, (q0 + n1 - 1) // 128))
                        n0 = n1
                    units.append((qh, h, kt, o, chunks))

    exp_tiles = {}
    emitted_se = set()

    SHORT_SMALL = opts.get("short_small", False)

    def emit_se(i):
        if i in emitted_se:
            return
        emitted_se.add(i)
        qh, h, kt, o, chunks = units[i]
        q0 = qh * QCH
        p0 = HD * (h % 2)
        c2 = h // 2
        # short diagonal units (S region within one 512-col bank) borrow the
        # small pool so long units get the full s_ps ring to themselves
        short = SHORT_SMALL and o >= QCH - 512
        if short:
            S = small_ps.tile([128, 512], f32, tag="sp", name="Ss")
            base = QCH - 512
        else:
            S = s_ps.tile([128, QCH], f32, tag="S", name="S")
            base = 0
        for (n0, n1, _) in chunks:
            MM_LABELS.append(f"S.q{qh}h{h}k{kt}.{n0}")
            nc.tensor.matmul(
                S[:, n0 - base : n1 - base],
                lhsT=kT8[p0 : p0 + HD, c2, :, 128 * kt : 128 * kt + 128],
                rhs=qT8[p0 : p0 + HD, c2, :, q0 + n0 : q0 + n1],
                start=True,
                stop=True,
                perf_mode=DR,
            )
        expS = expp.tile([128, QCH], bf16, tag="expS", name="expS")
        nc.scalar.activation(
            out=expS[:, o:QCH],
            in_=S[:, o - base : QCH - base],
            func=mybir.ActivationFunctionType.Exp,
            scale=INV_SQRT_D,
            bias=lnm_sb[:, kt : kt + 1],
        )
        if 128 * kt >= q0:
            if opts.get("tri_engine", "vector") == "pool":
                nc.gpsimd.tensor_mul(
                    out=expS[:, o : o + 128], in0=expS[:, o : o + 128],
                    in1=tri_sb)
            else:
                nc.vector.tensor_mul(
                    out=expS[:, o : o + 128], in0=expS[:, o : o + 128],
                    in1=tri_sb)
        exp_tiles[i] = expS

    def emit_finalize(qh, h, j0, j1, last=False):
        # normalize ctx columns [128*j0, 128*j1) of chunk qh for head h
        q0 = qh * QCH
        p0 = HD * (h % 2)
        c2 = h // 2
        n0, n1 = 128 * j0, 128 * j1
        w = n1 - n0
        ctx_t = cur_ctx[0]
        if last:
            # final bank: no next head waits on this PSUM tile -- normalize
            # straight out of PSUM, in 128-col pieces so the first tail wo
            # unblocks after ~0.8us instead of the full-bank chain
            for m0 in range(n0, n1, 128):
                m1 = m0 + 128
                mw = 128
                r = rp.tile([1, mw], bf16, tag="r", name="r")
                with nc.allow_low_precision(reason="softmax denom recip bf16"):
                    nc.vector.reciprocal(r, ctx_t[HD : HD + 1, m0:m1])
                r64 = rp.tile([HD, mw], bf16, tag="r64", name="r64")
                nc.gpsimd.partition_broadcast(r64, r)
                nc.vector.tensor_mul(
                    out=ctxT[c2][p0 : p0 + HD, q0 + m0 : q0 + m1],
                    in0=ctx_t[0:HD, m0:m1],
                    in1=r64,
                )
            return
        # copy PSUM->SBUF first (releases the ctx accumulator for the next
        # head), then normalize the SBUF copy; bf16 end-to-end so the DVE
        # 2x two-byte mode applies to recip and mul
        ctx_sb = ctxsbp.tile([HD + 1, w], bf16, tag="ctxsb", name="ctxsb")
        nc.vector.tensor_copy(out=ctx_sb, in_=ctx_t[0 : HD + 1, n0:n1])
        r = rp.tile([1, w], bf16, tag="r", name="r")
        with nc.allow_low_precision(reason="softmax denom recip in bf16"):
            nc.vector.reciprocal(r, ctx_sb[HD : HD + 1, :])
        r64 = rp.tile([HD, w], bf16, tag="r64", name="r64")
        nc.gpsimd.partition_broadcast(r64, r)
        nc.vector.tensor_mul(
            out=ctxT[c2][p0 : p0 + HD, q0 + n0 : q0 + n1],
            in0=ctx_sb[0:HD, :],
            in1=r64,
        )

    osb_tiles = {}

    def emit_wo(qh, lt, n2, evac="vector"):
        # both 512-col halves of a row-tile stage into one [128,1024] SBUF
        # tile and go out in a single 2KB-per-partition DMA (halves HWDGE
        # dispatch cost at the tail)
        q0 = qh * QCH
        l0 = q0 + 128 * lt
        wps = small_ps.tile([128, 512], f32, tag="sp", name="wops")
        for cc in range(2):
            MM_LABELS.append(f"wo.q{qh}l{lt}n{n2}.{cc}")
            nc.tensor.matmul(
                wps,
                lhsT=ctxT[cc][:, l0 : l0 + 128],
                rhs=wo_sb[:, cc, 512 * n2 : 512 * n2 + 512],
                start=(cc == 0),
                stop=(cc == 1),
            )
        if (qh, lt) not in osb_tiles:
            osb_tiles[(qh, lt)] = outp.tile([128, H], bf16, tag="osb", name="osb")
        osb = osb_tiles[(qh, lt)]
        if evac == "scalar":
            nc.scalar.copy(out=osb[:, 512 * n2 : 512 * n2 + 512], in_=wps)
        else:
            nc.vector.tensor_copy(out=osb[:, 512 * n2 : 512 * n2 + 512], in_=wps)
        if n2 == 1:
            del osb_tiles[(qh, lt)]
            nc.sync.dma_start(out=out_d[l0 : l0 + 128, :], in_=osb)

    # ---- build interleaved schedule ----
    # background injections keyed by attention-unit index
    inject = {i: [] for i in range(len(units) + 1)}
    for j in range(NLT):
        inject[min(V_START + j * V_STEP, len(units))].append(("v", j))
    k = 0
    for c2 in range(2):
        for qc in range(4):
            for which in ("q", "k"):
                if c2 == 0:
                    continue
                inject[min(QK1_START + k * QK1_STEP, len(units))].append(
                    ("qk", which, 1, qc))
                k += 1
    # wo(qh) becomes ready when the (qh, h3) group ends; spread its units
    # into the stream after that point (the last qh is handled by the tail)
    last_qh = NQH - 1
    h3_end = {}
    for i, (qh, h, kt, o, chunks) in enumerate(units):
        if h == 3:
            h3_end[qh] = i + 1
    for qh in range(NQH - 1):
        wo_units = [(qh, lt, n2) for lt in range(QCH // 128) for n2 in range(2)]
        for j, wu in enumerate(wo_units):
            ev = "scalar" if (opts.get("wo0_alt", False) and j % 2) else "vector"
            inject[min(h3_end[qh] + j * WO0_STEP, len(units))].append(
                ("wo", wu[0], wu[1], wu[2], ev))

    cur_ctx = [None]
    se_cursor = [0]
    tail_wo = []


    def se_ready(i):
        # S(i) may only be emitted after the projections it reads are emitted
        qh, h, kt, o, chunks = units[i]
        c2 = h // 2
        for qc in range(qh * (QCH // 512), (qh + 1) * (QCH // 512)):
            if ("q", c2, qc) not in emitted_proj:
                return False
        return ("k", c2, kt // 4) in emitted_proj

    def pump_se(limit):
        while se_cursor[0] <= min(limit, len(units) - 1) and se_ready(se_cursor[0]):
            emit_se(se_cursor[0])
            se_cursor[0] += 1

    pf_cursor = [len(units) // 2]  # start of the hb=1 half


    # minimal projection prefix: S(qh0,h0,kt0..3) needs q qc0-1 + k qc0;
    # everything else streams into the attention loop below
    if opts.get("full_prefix", True):
        PRE = [("q", 0, qc) for qc in range(4)] + [("k", 0, qc) for qc in range(4)]
    else:
        PRE = [("q", 0, 0), ("k", 0, 0), ("q", 0, 1)]
    for which, c2, qc in PRE:
        emit_qk(which, c2, qc)
    pre_set = set(PRE)
    # remaining c2=0 projections injected early (kt order needs k qc asc)
    rest = [it for it in
            [("k", 0, 1), ("q", 0, 2), ("k", 0, 2), ("q", 0, 3), ("k", 0, 3)]
            if it not in pre_set]
    QK0_START = opts.get("qk0_start", 1)
    QK0_STEP = opts.get("qk0_step", 2)
    for j, it in enumerate(rest):
        inject[min(QK0_START + j * QK0_STEP, len(units))].append(("qk",) + it)

    PUMP_FIRST = opts.get("pump_first", True)
    for i, (qh, h, kt, o, chunks) in enumerate(units):
        if PUMP_FIRST:
            pump_se(i + LA)
        # prefetch hb1 S/exp into the hb0 window (ACT has slack there)
        if (PF_MAX > 0 and i >= PF_START and (i - PF_START) % PF_STEP == 0
                and i < len(units) // 2
                and pf_cursor[0] < len(units) // 2 + PF_MAX
                and pf_cursor[0] > se_cursor[0]
                and se_ready(pf_cursor[0])):
            emit_se(pf_cursor[0])
            pf_cursor[0] += 1
        for item in inject.get(i, []):
            if item[0] == "v":
                emit_v(item[1])
            elif item[0] == "qk":
                emit_qk(item[1], item[2], item[3])
            elif item[0] == "wo":
                emit_wo(item[1], item[2], item[3], evac=item[4])
        pump_se(i + LA)
        q0 = qh * QCH
        c2 = h // 2
        if kt == 0:
            cur_ctx[0] = ctx_ps.tile([128, QCH], f32, tag="ctx", name="ctx")
        expS = exp_tiles.pop(i)
        for (n0, n1, kstop) in chunks:
            MM_LABELS.append(f"PV.q{qh}h{h}k{kt}.{n0}")
            nc.tensor.matmul(
                cur_ctx[0][0 : HD + 1, n0:n1],
                lhsT=v_aug[:, kt, (HD + 1) * h : (HD + 1) * (h + 1)],
                rhs=expS[:, n0:n1],
                start=(kt == 0),
                stop=(kt == kstop),
            )
        ktmax = (q0 + QCH - 1) // 128
        tail_head = TAIL_SPLIT and qh == last_qh and h == 3
        if tail_head:
            # drain queued wo BEFORE this unit's finalize: drains then lag
            # the finalize chain by one unit so its DVE mul has finished
            for _ in range(opts.get("tail_wo_rate", 2)):
                if tail_wo:
                    tqh, tlt, tn2 = tail_wo.pop(0)
                    emit_wo(tqh, tlt, tn2,
                            evac=("scalar" if tn2 else "vector"))
            # PSUM accumulation groups close per 2KB bank (512 f32 cols):
            # finalize bank b as soon as its last PV lands, then trickle its
            # wo units between the remaining PV units so the PSUM ring and
            # evac engines pipeline instead of stalling PE back-to-back
            for b in range(QCH // 512):
                if kt == (q0 + 512 * (b + 1) - 1) // 128:
                    emit_finalize(qh, h, 4 * b, 4 * b + 4,
                                  last=(b == QCH // 512 - 1))
                    for lt in range(4 * b, 4 * b + 4):
                        for n2 in range(2):
                            tail_wo.append((qh, lt, n2))
        elif kt == ktmax:
            emit_finalize(qh, h, 0, QCH // 128)
            if qh == last_qh and h == 3:
                for lt in range(QCH // 128):
                    for n2 in range(2):
                        emit_wo(qh, lt, n2,
                                evac=("scalar" if n2 else "vector"))
    while tail_wo:
        tqh, tlt, tn2 = tail_wo.pop(0)
        emit_wo(tqh, tlt, tn2, evac=("scalar" if tn2 else "vector"))
    for item in inject.get(len(units), []):
        if item[0] == "v":
            emit_v(item[1])
        elif item[0] == "qk":
            emit_qk(item[1], item[2], item[3])
        elif item[0] == "wo":
            emit_wo(item[1], item[2], item[3], evac=item[4])


def _get_program(L=2048, QCH=1024):
    key = (L, QCH)
    if key not in _PROGRAM_CACHE:
        _PROGRAM_CACHE[key] = build_program(L, QCH)
    return _PROGRAM_CACHE[key]


def make_in_maps(x, am, wq, wk, wv, wo):
    B, L, _ = x.shape
    tri = np.triu(np.ones((128, 128), dtype=np.float32)).astype(ml_dtypes.bfloat16)
    in_maps = []
    for c in range(8):
        b, g = divmod(c, 4)
        cols = slice(WG * g, WG * (g + 1))
        m = am[b].astype(np.float32)
        lnm = np.where(m > 0, np.log(np.maximum(m, 1e-38)), -1e38).astype(np.float32)
        onescol = np.ones((128, (L // 128) * 4), dtype=ml_dtypes.bfloat16)
        xT = np.ascontiguousarray(x[b].T)
        in_maps.append(
            {
                "onescol": onescol,
                "xTb": xT.astype(ml_dtypes.bfloat16),
                "wqb": np.ascontiguousarray(wq[:, cols]).astype(ml_dtypes.bfloat16),
                "wkb": np.ascontiguousarray(wk[:, cols]).astype(ml_dtypes.bfloat16),
                "wvb": np.ascontiguousarray(wv[:, cols]).astype(ml_dtypes.bfloat16),
                "wob": np.ascontiguousarray(wo[cols, :]).astype(ml_dtypes.bfloat16),
                "lnm": lnm,
                "tri": tri,
            }
        )
    return in_maps


def kernel(**inputs):
    global LAST_RESULT
    x = np.asarray(inputs["input"], dtype=np.float32)
    am = np.asarray(inputs["attention_mask"], dtype=np.float32)
    wq = np.asarray(inputs["wq"], dtype=np.float32)
    wk = np.asarray(inputs["wk"], dtype=np.float32)
    wv = np.asarray(inputs["wv"], dtype=np.float32)
    wo = np.asarray(inputs["wo"], dtype=np.float32)
    B, L, _ = x.shape

    nc = _get_program(L=L, QCH=min(1024, L))
    in_maps = make_in_maps(x, am, wq, wk, wv, wo)
    trace = os.environ.get("KERNEL_TRACE", "0") == "1"
    res = run_bass_kernel_spmd(nc, in_maps, list(range(8)), trace=trace)
    LAST_RESULT = res

    out = np.zeros((B, L, H), dtype=np.float32)
    for b in range(B):
        for g in range(4):
            out[b] += res.results[4 * b + g]["out"].astype(np.float32)
    return out
